# revision 1
# baseline (speedup 1.0000x reference)
"""CCA few-shot scoring kernel for Trainium2 (8 NeuronCores, SPMD).

Inputs (full): spt [1,5,3840,5,5] f32, qry [75,3840,5,5] f32.
Output: sim [75,5] f32.

Sharding: data-parallel over queries. 75 queries padded to 80; each of the
8 cores handles 10 queries against the full replicated support set.

Per-core pipeline (q=10 local queries, way=5, C=3840, 25 spatial positions,
levels d in {256,512,1024,2048}; all data staged bf16, partition-major):
  1. channel means via PE ones-chains (overlapped with DMA), in-place
     centering split across DVE/Pool
  2. per level, PE chains: corr [125,250] = S0^T Q0, GramS [125,125],
     GramQ halves [125,125]x2, accumulating over 128-channel chunks
  3. softmax attention:
     - gaussian-normalize mean subtraction cancels in softmax (shift
       invariance); only the 1/std scale matters
     - the l2-normalizations fold per softmax: t-softmax only needs
       corr*inv_nq, s-softmax only needs corr*inv_ns (gn scale invariance)
     - t-softmax stats via narrow DVE reduces [125,10]; s-softmax stats and
       all cross-partition broadcasts/sums via PE indicator matmuls
     - all rsqrt as exp(-0.5*ln(x)): ln/exp share one activation table, so
       zero 1283ns table reloads
  4. dot = attn_s^T corr attn_q, |pooled|^2 via masked-gram quadratic
     forms; accumulated over levels in one PSUM bank; cosine /0.2.
"""

import json

import numpy as np
from concourse import bass, mybir
from concourse import bass2jax as _b2j
from concourse import bass_utils as _bu
from concourse.tile import TileContext
from concourse.bass_utils import run_bass_kernel_spmd


def _split_multiwaits(bir_json: bytes) -> bytes:
    """Walrus in this env allows one sync-wait per compute instruction.

    Split every multi-wait instruction: hoist all but the last wait onto
    fresh single-wait RegisterMove carriers (same engine, immediately
    preceding), cloned from the preamble zero-reg writes.
    """
    j = json.loads(bir_json)
    tmpl = {}
    for f in j["functions"]:
        for b in f["blocks"]:
            for i in b["instructions"]:
                if i["opcode"] == "RegisterMove":
                    for o in i.get("outs", []):
                        if str(o.get("regref", "")).endswith("_zero"):
                            tmpl.setdefault(i["engine"], i)
    uid = [0]

    def carrier(engine, wait):
        t = tmpl[engine]
        uid[0] += 1
        return {
            "debug": t.get("debug", 0),
            "engine": engine,
            "ins": [dict(x) for x in t["ins"]],
            "name": f"IW-{uid[0]}",
            "opcode": "RegisterMove",
            "outs": [dict(x) for x in t["outs"]],
            "sync_info": {"on_update": [], "on_wait": [wait]},
        }

    for f in j["functions"]:
        for b in f["blocks"]:
            out = []
            for i in b["instructions"]:
                si = i.get("sync_info")
                ow = si.get("on_wait") if si else None
                if ow and len(ow) > 1:
                    for w in ow[:-1]:
                        out.append(carrier(i["engine"], w))
                    si["on_wait"] = [ow[-1]]
                out.append(i)
            b["instructions"] = out
    return json.dumps(j).encode()


_orig_cbk = _bu.compile_bir_kernel


def _patched_cbk(bir_json, tmpdir, neff_name="file.neff"):
    return _orig_cbk(_split_multiwaits(bir_json), tmpdir, neff_name=neff_name)


for _mod in (_b2j, _bu):
    if getattr(_mod, "compile_bir_kernel", None) is _orig_cbk:
        _mod.compile_bir_kernel = _patched_cbk

F32 = mybir.dt.float32
BF16 = mybir.dt.bfloat16
AX = mybir.AxisListType.X
ADD = mybir.AluOpType.add
MUL = mybir.AluOpType.mult
EXP = mybir.ActivationFunctionType.Exp
LN = mybir.ActivationFunctionType.Ln
COPY = mybir.ActivationFunctionType.Copy
SQUARE = mybir.ActivationFunctionType.Square

HYPER = [256, 512, 1024, 2048]
C = 3840
WAY = 5
SS = 25           # fs*fs
NQ = 75
NQL = 10          # queries per core
NCORES = 8
P = 128
NCH = C // P      # 30 channel chunks
WS = WAY * SS     # 125
QT = NQL * SS     # 250
T_ATTN = 5.0
L2_EPS = 1e-6
GN_EPS = 1e-5
NLV = 4
LCH = [d // P for d in HYPER]                 # chunks per level: 2,4,8,16

# bf16 const image columns: I125 | WMASK | BIND | BINDT | ONESR | ONESC
C_I = 0
C_WM = 125
C_BI = 250
C_BIT = 255
C_OR = 380
C_OC = 508
NCB = 509
# f32 const image: IF32
NCF = 125

_CACHE = {}
LINEARIZE = False
DEBUG_MARKS = False


def _mark(nc, label, ap=None):
    if not DEBUG_MARKS or ap is None:
        return
    from concourse import bass_interp

    def cb(sim, inst, _l=label):
        print(f"  [mark] {_l:28s} t={sim.time:8.0f}")
    bass_interp.add_callback2(nc.gpsimd, cb, ins=[ap])


def _build_nc():
    nc = bass.Bass()
    for val in (L2_EPS, GN_EPS, (SS - 1) * GN_EPS,
                0.5 * float(np.log(SS - 1)), float(np.log(5.0))):
        t = nc.alloc_sbuf_tensor(f"const-f32-{val}", [128, 1], F32)
        nc.gpsimd.memset(t.ap(), val)
        nc.const_aps.aps[(F32, val)] = t.ap()
    nc.all_engine_barrier()

    qd = [nc.declare_dram_parameter(f"q{l}", [P, LCH[l] * QT], BF16,
                                    isOutput=False) for l in range(NLV)]
    sd = [nc.declare_dram_parameter(f"s{l}", [P, LCH[l] * WS], BF16,
                                    isOutput=False) for l in range(NLV)]
    cb_d = nc.declare_dram_parameter("cb", [P, NCB], BF16, isOutput=False)
    cf_d = nc.declare_dram_parameter("cf", [P, NCF], F32, isOutput=False)
    out_d = nc.declare_dram_parameter("out", [WAY, NQL], F32, isOutput=True)

    with TileContext(nc, linearize=LINEARIZE) as tc:
        with (
            tc.tile_pool(name="const", bufs=1) as cpool,
            tc.tile_pool(name="data", bufs=1) as data,
            tc.tile_pool(name="mean", bufs=1) as mean,
            tc.tile_pool(name="big", bufs=4) as big,
            tc.tile_pool(name="small", bufs=4) as small,
            tc.tile_pool(name="ps_chain", bufs=1, space="PSUM") as ps_chain,
            tc.tile_pool(name="ps_work", bufs=3, space="PSUM") as ps_work,
            tc.tile_pool(name="ps_acc", bufs=1, space="PSUM") as ps_acc,
        ):
            # ---- constants ----
            CB = cpool.tile([P, NCB], BF16)
            nc.sync.dma_start(out=CB[:, :], in_=cb_d[:, :])
            CF = cpool.tile([P, NCF], F32)
            nc.sync.dma_start(out=CF[:, :], in_=cf_d[:, :])
            I125 = CB[0:WS, C_I:C_I + 125]
            WMASK = CB[0:WS, C_WM:C_WM + 125]
            BIND = CB[0:WS, C_BI:C_BI + 5]
            BINDT = CB[0:WAY, C_BIT:C_BIT + 125]
            ONESR = CB[0:1, C_OR:C_OR + 128]
            ONESR125 = CB[0:1, C_OR:C_OR + 125]
            ONESC = CB[0:P, C_OC:C_OC + 1]
            IF32 = CF[0:WS, 0:125]
            IF5 = CF[0:WAY, 0:WAY]

            # ---- data loads (one DMA per level per tensor) ----
            qt = [data.tile([P, LCH[l] * QT], BF16, tag=f"q{l}",
                            name=f"q{l}") for l in range(NLV)]
            st = [data.tile([P, LCH[l] * WS], BF16, tag=f"s{l}",
                            name=f"s{l}") for l in range(NLV)]
            qeng = [nc.sync, nc.scalar, nc.gpsimd]
            for l in range(NLV):
                qeng[(2 * l) % 3].dma_start(out=st[l][:, :], in_=sd[l][:, :])
                qeng[(2 * l + 1) % 3].dma_start(out=qt[l][:, :],
                                                in_=qd[l][:, :])

            # ---- pair-sums (DVE/Pool, overlap DMA) then PE mean chains ----
            qp = [data.tile([P, (LCH[l] // 2) * QT], BF16, tag=f"qp{l}",
                            name=f"qp{l}") for l in range(NLV)]
            sp = [data.tile([P, (LCH[l] // 2) * WS], BF16, tag=f"sp{l}",
                            name=f"sp{l}") for l in range(NLV)]
            flip = [0]

            def _pair(dst, a, b, width, npair):
                d3 = dst.rearrange("p (k t) -> p k t", t=width)
                a3 = a.rearrange("p (k t) -> p k t", t=width)
                b3 = b.rearrange("p (k t) -> p k t", t=width)
                eng = nc.vector if flip[0] % 2 == 0 else nc.gpsimd
                flip[0] += 1
                eng.tensor_add(d3, a3, b3)

            for l in range(NLV):
                h = LCH[l] // 2
                _pair(qp[l][:, :], qt[l][:, 0:h * QT], qt[l][:, h * QT:],
                      QT, h)
                _pair(sp[l][:, :], st[l][:, 0:h * WS], st[l][:, h * WS:],
                      WS, h)

            ps_mq = ps_work.tile([1, QT], F32, tag="pw")
            ps_ms = ps_work.tile([1, WS], F32, tag="pw")
            nmm = sum(LCH) // 2
            i = 0
            for l in range(NLV):
                for k in range(LCH[l] // 2):
                    nc.tensor.matmul(ps_ms[:, :], ONESC,
                                     sp[l][:, k * WS:(k + 1) * WS],
                                     start=(i == 0), stop=(i == nmm - 1))
                    nc.tensor.matmul(ps_mq[:, :], ONESC,
                                     qp[l][:, k * QT:(k + 1) * QT],
                                     start=(i == 0), stop=(i == nmm - 1))
                    i += 1
            # negated means -> bf16 rows
            nmq = mean.tile([1, QT], BF16, name="nmq")
            nc.vector.tensor_scalar_mul(nmq[:, :], ps_mq[:, :], -1.0 / C)
            nms = mean.tile([1, WS], BF16, name="nms")
            nc.vector.tensor_scalar_mul(nms[:, :], ps_ms[:, :], -1.0 / C)
            # broadcast to all partitions
            ps_mqb = ps_work.tile([P, QT], F32, tag="pw")
            nc.tensor.matmul(ps_mqb[:, :], ONESR, nmq[:, :],
                             start=True, stop=True)
            nmq_bc = mean.tile([P, QT], BF16, name="nmqbc")
            nc.vector.tensor_copy(nmq_bc[:, :], ps_mqb[:, :])
            ps_msb = ps_work.tile([P, WS], F32, tag="pw")
            nc.tensor.matmul(ps_msb[:, :], ONESR, nms[:, :],
                             start=True, stop=True)
            nms_bc = mean.tile([P, WS], BF16, name="nmsbc")
            nc.vector.tensor_copy(nms_bc[:, :], ps_msb[:, :])
            _mark(nc, "means-done", nms_bc[:, :])

            def _center(l):
                m = LCH[l]
                # DVE gets ~55% of columns (bf16 2x), Pool the rest
                for tile, width in ((qt[l], QT), (st[l], WS)):
                    hD = (m * 55 + 50) // 100
                    hD = max(1, min(m - 1, hD))
                    bc = nmq_bc if width == QT else nms_bc
                    nc.vector.tensor_add(
                        tile[:, 0:hD * width].rearrange(
                            "p (k t) -> p k t", t=width),
                        tile[:, 0:hD * width].rearrange(
                            "p (k t) -> p k t", t=width),
                        bc[:, :].unsqueeze(1).to_broadcast([P, hD, width]))
                    nc.gpsimd.tensor_add(
                        tile[:, hD * width:].rearrange(
                            "p (k t) -> p k t", t=width),
                        tile[:, hD * width:].rearrange(
                            "p (k t) -> p k t", t=width),
                        bc[:, :].unsqueeze(1).to_broadcast([P, m - hD,
                                                            width]))
            _mark(nc, "center-done", qt[3][:, 0:1])

            # ---- accumulators: one PSUM bank [5, 30] ----
            # cols 0:10 dot [w,q], 10:20 s2 [w,q], 20:25 q2h0 [q,w], 25:30 q2h1
            acc = ps_acc.tile([WAY, 30], F32, name="acc")
            first_acc = [True]
            n_accmm = 4 * NLV
            i_accmm = [0]

            def acc_mm(lhs, rhs, c0, c1):
                nc.tensor.matmul(acc[:, c0:c1], lhs, rhs,
                                 start=first_acc[0],
                                 stop=i_accmm[0] == n_accmm - 1)
                first_acc[0] = False
                i_accmm[0] += 1

            # ---- all chains first (dense PE stream, ramps pstate) ----
            # per level: bank A = corr(0:250)+gramS(250:375),
            #            bank B = gramQ0(0:125)+gramQ1(125:250)
            chain_ps = []
            for l in range(NLV):
                m = LCH[l]
                _center(l)
                chA = ps_chain.tile([WS, 512], F32, tag=f"chA{l % 2}")
                chB = ps_chain.tile([WS, 512], F32, tag=f"chB{l % 2}")
                chain_ps.append((chA, chB))
                for k in range(m):
                    sk = st[l][:, k * WS:(k + 1) * WS]
                    qk = qt[l][:, k * QT:(k + 1) * QT]
                    s0, sp = (k == 0), (k == m - 1)
                    nc.tensor.matmul(chA[:, 0:QT], sk, qk, start=s0,
                                     stop=False)
                    nc.tensor.matmul(chA[:, QT:QT + WS], sk, sk, start=False,
                                     stop=sp)
                    nc.tensor.matmul(chB[:, 0:WS], qk[:, 0:WS],
                                     qk[:, 0:WS], start=s0, stop=False)
                    nc.tensor.matmul(chB[:, WS:2 * WS],
                                     qk[:, WS:QT], qk[:, WS:QT], start=False,
                                     stop=sp)

            # ---- per-level post phases, software-pipelined by stages ----
            V = [dict() for _ in range(NLV)]
            _rot = [0]
            _banks = [(ps_work, "pw"), (ps_chain, "chA0"),
                      (ps_chain, "chB0"), (ps_work, "pw"),
                      (ps_chain, "chA1"), (ps_chain, "chB1"),
                      (ps_work, "pw")]

            def pwork(shape):
                pool, tag = _banks[_rot[0] % len(_banks)]
                _rot[0] += 1
                return pool.tile(shape, F32, tag=tag,
                                 name=f"pw{_rot[0]}")

            def st0(l, v):
                chA, chB = chain_ps[l]
                cg = big.tile([WS, QT + WS], BF16, tag="cg")
                nc.scalar.activation(cg[:, :], chA[:, 0:QT + WS], COPY)
                v["c2"] = cg[:, 0:QT]
                v["gs16"] = cg[:, QT:QT + WS]
                gq16 = big.tile([WS, 2 * WS], BF16, tag="gq16")
                nc.vector.tensor_copy(gq16[:, :], chB[:, 0:2 * WS])
                v["gq16"] = gq16

            def st1(l, v):
                c2, gs16, gq16 = v["c2"], v["gs16"], v["gq16"]
                csq = big.tile([WS, QT], BF16, tag="csq")
                nc.gpsimd.tensor_mul(csq[:, :], c2, c2)
                v["csq"] = csq
                g3 = big.tile([WS, 3 * WS], BF16, tag="g3")
                nc.gpsimd.tensor_mul(g3[:, 0:WS], gs16, I125)
                nc.gpsimd.tensor_mul(g3[:, WS:2 * WS], gq16[:, 0:WS], I125)
                nc.gpsimd.tensor_mul(g3[:, 2 * WS:3 * WS], gq16[:, WS:2 * WS],
                                     I125)
                # diag sums via PE: ns^2 as a column, nq^2 as a row
                ps_nsc = pwork([WS, 1])
                nc.tensor.matmul(ps_nsc[:, :], g3[:, 0:WS], ONESC[0:WS, :],
                                 start=True, stop=True)
                v["ps_nsc"] = ps_nsc
                ps_nqr = pwork([1, QT])
                nc.tensor.matmul(ps_nqr[:, 0:WS], ONESC[0:WS, :],
                                 g3[:, WS:2 * WS], start=True, stop=False)
                nc.tensor.matmul(ps_nqr[:, WS:QT], ONESC[0:WS, :],
                                 g3[:, 2 * WS:3 * WS], start=False, stop=True)
                v["ps_nqr"] = ps_nqr
                gsm = big.tile([WS, WS], BF16, tag="gsm")
                nc.gpsimd.tensor_mul(gsm[:, :], gs16, WMASK)
                v["gsm"] = gsm
                g0m = big.tile([WS, WS], BF16, tag="g0m")
                nc.gpsimd.tensor_mul(g0m[:, :], gq16[:, 0:WS], WMASK)
                v["g0m"] = g0m
                g1m = big.tile([WS, WS], BF16, tag="g1m")
                nc.gpsimd.tensor_mul(g1m[:, :], gq16[:, WS:2 * WS], WMASK)
                v["g1m"] = g1m

            def st2(l, v):
                invns = small.tile([WS, 1], F32, tag="invns")
                nc.scalar.activation(invns[:, :], v["ps_nsc"][:, :], LN,
                                     bias=L2_EPS)
                nc.scalar.activation(invns[:, :], invns[:, :], EXP,
                                     scale=-0.5)
                v["invns"] = invns
                nqrf = small.tile([1, QT], F32, tag="nqrf")
                nc.scalar.activation(nqrf[:, :], v["ps_nqr"][:, :], LN,
                                     bias=L2_EPS)
                nqr = small.tile([1, QT], BF16, tag="nqr")
                nc.scalar.activation(nqr[:, :], nqrf[:, :], EXP, scale=-0.5)
                v["nqr"] = nqr
                ins5 = small.tile([WS, 1], F32, tag="ins5")
                nc.vector.tensor_scalar_mul(ins5[:, :], invns[:, 0:1],
                                            1.0 / SS)
                wns = small.tile([WS, WAY], BF16, tag="wns")
                nc.vector.tensor_scalar_mul(wns[:, :], BIND, ins5[:, 0:1])
                v["wns"] = wns
                ins2 = small.tile([WS, 1], F32, tag="ins2")
                nc.vector.tensor_mul(ins2[:, :], invns[:, 0:1], invns[:, 0:1])
                wns2 = small.tile([WS, WAY], BF16, tag="wns2")
                nc.vector.tensor_scalar_mul(wns2[:, :], BIND, ins2[:, 0:1])
                v["wns2"] = wns2
                insT = small.tile([WS, 1], F32, tag="insT")
                nc.vector.tensor_scalar_mul(insT[:, :], invns[:, 0:1],
                                            1.0 / T_ATTN)
                v["insT"] = insT

            def st3(l, v):
                c2, csq = v["c2"], v["csq"]
                ps_nqb = pwork([WS, QT])
                nc.tensor.matmul(ps_nqb[:, :], ONESR125, v["nqr"][:, :],
                                 start=True, stop=True)
                nqb16 = big.tile([WS, QT], BF16, tag="nqb16")
                nc.scalar.activation(nqb16[:, :], ps_nqb[:, :], COPY)
                y = big.tile([WS, QT], BF16, tag="y")
                nc.gpsimd.tensor_mul(y[:, :], c2, nqb16[:, :])
                v["y"] = y
                y3 = y[:, :].rearrange("p (g t) -> p g t", t=SS)
                s1 = small.tile([WS, NQL], F32, tag="s1")
                nc.vector.tensor_reduce(s1[:, :], y3, AX, ADD)
                v["s1"] = s1
                ysq = big.tile([WS, QT], BF16, tag="ysq")
                nc.gpsimd.tensor_mul(ysq[:, :], y[:, :], y[:, :])
                s2 = small.tile([WS, NQL], F32, tag="s2")
                nc.vector.tensor_reduce(
                    s2[:, :], ysq[:, :].rearrange("p (g t) -> p g t", t=SS),
                    AX, ADD)
                v["s2"] = s2
                ps_s1s = pwork([WAY, QT])
                nc.tensor.matmul(ps_s1s[:, :], v["wns"][:, :], c2,
                                 start=True, stop=True)
                ps_s2s = pwork([WAY, QT])
                nc.tensor.matmul(ps_s2s[:, :], v["wns2"][:, :], csq[:, :],
                                 start=True, stop=True)
                s1s16 = big.tile([WAY, QT], BF16, tag="s1s16")
                nc.scalar.activation(s1s16[:, :], ps_s1s[:, :], COPY)
                s1s_sq = big.tile([WAY, QT], F32, tag="s1ssq")
                nc.gpsimd.tensor_mul(s1s_sq[:, :], s1s16[:, :], s1s16[:, :])
                vs = big.tile([WAY, QT], F32, tag="vs")
                nc.vector.tensor_sub(vs[:, :], ps_s2s[:, :], s1s_sq[:, :])
                v["vs"] = vs

            def st4(l, v):
                s1, s2 = v["s1"], v["s2"]
                vall = small.tile([WS, 2 * NQL], F32, tag="vall")
                s1sq = small.tile([WS, NQL], F32, tag="s1sq")
                nc.vector.tensor_mul(s1sq[:, :], s1[:, :], s1[:, :])
                nc.vector.tensor_scalar_mul(s1sq[:, :], s1sq[:, :], 1.0 / SS)
                nc.vector.tensor_sub(vall[:, 0:NQL], s2[:, :], s1sq[:, :])
                ps_vt = pwork([WS, NQL])
                nc.tensor.matmul(ps_vt[:, 0:WAY], v["vs"][:, 0:WS], IF5,
                                 is_transpose=True, start=True, stop=False)
                nc.tensor.matmul(ps_vt[:, WAY:NQL], v["vs"][:, WS:QT], IF5,
                                 is_transpose=True, start=False, stop=True)
                nc.scalar.activation(vall[:, NQL:2 * NQL], ps_vt[:, :], COPY)
                ivs = small.tile([WS, 2 * NQL], F32, tag="ivs")
                nc.scalar.activation(ivs[:, :], vall[:, :], LN,
                                     bias=(SS - 1) * GN_EPS)
                nc.scalar.activation(ivs[:, :], ivs[:, :], EXP, scale=-0.5,
                                     bias=0.5 * float(np.log(SS - 1)))
                v["ivs"] = ivs

            def st5(l, v):
                c2, y, ivs = v["c2"], v["y"], v["ivs"]
                z = big.tile([WS, QT], BF16, tag="z")
                nc.gpsimd.tensor_tensor(
                    z[:, :].rearrange("p (g t) -> p g t", t=SS),
                    y[:, :].rearrange("p (g t) -> p g t", t=SS),
                    ivs[:, 0:NQL].unsqueeze(2).to_broadcast([WS, NQL, SS]),
                    MUL)
                e = big.tile([WS, QT], BF16, tag="e")
                nc.scalar.activation(e[:, :], z[:, :], EXP, scale=1.0 / T_ATTN)
                v["e"] = e
                den = small.tile([WS, NQL], F32, tag="den")
                nc.vector.tensor_reduce(
                    den[:, :], e[:, :].rearrange("p (g t) -> p g t", t=SS),
                    AX, ADD)
                rden = small.tile([WS, NQL], F32, tag="rden")
                nc.vector.reciprocal(rden[:, :], den[:, :])
                v["rden"] = rden
                # s-softmax broadcast of invstd
                ps_ivT = pwork([WAY, QT])
                nc.tensor.matmul(ps_ivT[:, 0:WS], ivs[:, NQL:NQL + WAY], IF32,
                                 is_transpose=True, start=True, stop=False)
                nc.tensor.matmul(ps_ivT[:, WS:QT], ivs[:, NQL + WAY:2 * NQL],
                                 IF32, is_transpose=True, start=False,
                                 stop=True)
                ivs16 = big.tile([WAY, QT], BF16, tag="ivs16")
                nc.scalar.activation(ivs16[:, :], ps_ivT[:, :], COPY)
                ps_ivb = pwork([WS, QT])
                nc.tensor.matmul(ps_ivb[:, :], BINDT, ivs16[:, :],
                                 start=True, stop=True)
                zb = big.tile([WS, QT], BF16, tag="zb")
                nc.vector.tensor_mul(zb[:, :], c2, ps_ivb[:, :])
                es = big.tile([WS, QT], BF16, tag="es")
                nc.scalar.activation(es[:, :], zb[:, :], EXP,
                                     scale=v["insT"][:, 0:1])
                v["es"] = es

            def st6(l, v):
                e, rden, es = v["e"], v["rden"], v["es"]
                f = big.tile([WS, QT], BF16, tag="f")
                nc.gpsimd.tensor_tensor(
                    f[:, :].rearrange("p (g t) -> p g t", t=SS),
                    e[:, :].rearrange("p (g t) -> p g t", t=SS),
                    rden[:, :].unsqueeze(2).to_broadcast([WS, NQL, SS]), MUL)
                ps_aq = pwork([WAY, QT])
                nc.tensor.matmul(ps_aq[:, :], BIND, f[:, :],
                                 start=True, stop=True)
                aq16 = big.tile([WAY, QT], BF16, tag="aq16")
                nc.vector.tensor_copy(aq16[:, :], ps_aq[:, :])
                v["aq16"] = aq16
                aqf = big.tile([WAY, QT], F32, tag="aqf")
                nc.scalar.activation(aqf[:, :], ps_aq[:, :], COPY)
                v["aqf"] = aqf
                ps_dens = pwork([WAY, QT])
                nc.tensor.matmul(ps_dens[:, :], BIND, es[:, :],
                                 start=True, stop=True)
                rdens = big.tile([WAY, QT], BF16, tag="rdens")
                with nc.allow_low_precision(reason="softmax denom bf16"):
                    nc.vector.reciprocal(rdens[:, :], ps_dens[:, :])
                ps_rdb = pwork([WS, QT])
                nc.tensor.matmul(ps_rdb[:, :], BINDT, rdens[:, :],
                                 start=True, stop=True)
                rdb16 = big.tile([WS, QT], BF16, tag="rdb16")
                nc.scalar.activation(rdb16[:, :], ps_rdb[:, :], COPY)
                fs = big.tile([WS, QT], BF16, tag="fs")
                nc.gpsimd.tensor_mul(fs[:, :], es[:, :], rdb16[:, :])
                As = small.tile([WS, NQL], F32, tag="As")
                nc.vector.tensor_reduce(
                    As[:, :], fs[:, :].rearrange("p (g t) -> p g t", t=SS),
                    AX, ADD)
                v["As"] = As
                As16 = small.tile([WS, NQL], BF16, tag="As16")
                nc.gpsimd.tensor_copy(As16[:, :], As[:, :])
                v["As16"] = As16

            def st7(l, v):
                c2, As, As16 = v["c2"], v["As"], v["As16"]
                ps_aqb = pwork([WS, QT])
                nc.tensor.matmul(ps_aqb[:, :], BINDT, v["aq16"][:, :],
                                 start=True, stop=True)
                aqb16 = big.tile([WS, QT], BF16, tag="aqb16")
                nc.scalar.activation(aqb16[:, :], ps_aqb[:, :], COPY)
                u = big.tile([WS, QT], BF16, tag="u")
                nc.gpsimd.tensor_mul(u[:, :], c2, aqb16[:, :])
                vv = small.tile([WS, NQL], F32, tag="v")
                nc.vector.tensor_reduce(
                    vv[:, :], u[:, :].rearrange("p (g t) -> p g t", t=SS),
                    AX, ADD)
                zd = small.tile([WS, NQL], BF16, tag="zd")
                nc.vector.tensor_mul(zd[:, :], As[:, :], vv[:, :])
                acc_mm(BIND, zd[:, :], 0, NQL)
                ps_py = pwork([WS, NQL])
                nc.tensor.matmul(ps_py[:, :], v["gsm"][:, :], As16[:, :],
                                 start=True, stop=True)
                zz = small.tile([WS, NQL], BF16, tag="zz")
                nc.vector.tensor_mul(zz[:, :], As[:, :], ps_py[:, :])
                acc_mm(BIND, zz[:, :], NQL, 2 * NQL)

            def st8(l, v):
                aqf = v["aqf"]
                ps_aqT = pwork([WS, NQL])
                nc.tensor.matmul(ps_aqT[:, 0:WAY], aqf[:, 0:WS], IF5,
                                 is_transpose=True, start=True, stop=False)
                nc.tensor.matmul(ps_aqT[:, WAY:NQL], aqf[:, WS:QT], IF5,
                                 is_transpose=True, start=False, stop=True)
                aqT16 = small.tile([WS, NQL], BF16, tag="aqT16")
                nc.scalar.activation(aqT16[:, :], ps_aqT[:, :], COPY)
                ps_pz = pwork([WS, NQL])
                nc.tensor.matmul(ps_pz[:, 0:WAY], v["g0m"][:, :],
                                 aqT16[:, 0:WAY], start=True, stop=False)
                nc.tensor.matmul(ps_pz[:, WAY:NQL], v["g1m"][:, :],
                                 aqT16[:, WAY:NQL], start=False, stop=True)
                zq = small.tile([WS, NQL], BF16, tag="zq")
                nc.vector.tensor_mul(zq[:, :], aqT16[:, :], ps_pz[:, :])
                acc_mm(BIND, zq[:, 0:WAY], 2 * NQL, 2 * NQL + WAY)
                acc_mm(BIND, zq[:, WAY:NQL], 2 * NQL + WAY, 30)
                _mark(nc, f"level{l}-done", zq[:, :])

            stages = [st0, st1, st2, st3, st4, st5, st6, st7, st8]
            for s_fn in stages:
                for l in range(NLV):
                    s_fn(l, V[l])

            # ---- final cosine ----
            accs = small.tile([WAY, 30], F32, tag="accs")
            nc.vector.tensor_copy(accs[:, :], acc[:, :])
            ps_q2t = ps_work.tile([WAY, NQL], F32, tag="pw")
            nc.tensor.matmul(ps_q2t[:, 0:WAY], accs[:, 20:25], IF5,
                             is_transpose=True, start=True, stop=False)
            nc.tensor.matmul(ps_q2t[:, WAY:NQL], accs[:, 25:30], IF5,
                             is_transpose=True, start=False, stop=True)
            den2 = small.tile([WAY, NQL], F32, tag="den2")
            nc.vector.tensor_mul(den2[:, :], accs[:, NQL:2 * NQL],
                                 ps_q2t[:, :])
            invd = small.tile([WAY, NQL], F32, tag="invd")
            nc.scalar.activation(invd[:, :], den2[:, :], LN)
            nc.scalar.activation(invd[:, :], invd[:, :], EXP, scale=-0.5,
                                 bias=float(np.log(5.0)))
            sim = small.tile([WAY, NQL], F32, tag="sim")
            nc.vector.tensor_mul(sim[:, :], accs[:, 0:NQL], invd[:, :])
            nc.sync.dma_start(out=out_d[:, :], in_=sim[:, :])
    return nc


def _constants():
    i125 = np.eye(WS, dtype=np.float32)
    wmask = np.kron(np.eye(WAY, dtype=np.float32),
                    np.ones((SS, SS), dtype=np.float32))
    bind = np.zeros((WS, WAY), dtype=np.float32)
    for w in range(WAY):
        bind[w * SS:(w + 1) * SS, w] = 1.0
    cb = np.zeros((P, NCB), dtype=np.float32)
    cb[0:WS, C_I:C_I + 125] = i125
    cb[0:WS, C_WM:C_WM + 125] = wmask
    cb[0:WS, C_BI:C_BI + 5] = bind
    cb[0:WAY, C_BIT:C_BIT + 125] = bind.T
    cb[0:1, C_OR:C_OR + 128] = 1.0
    cb[0:P, C_OC:C_OC + 1] = 1.0
    cf = np.zeros((P, NCF), dtype=np.float32)
    cf[0:WS, 0:125] = i125
    import jax.numpy as jnp
    return {
        "cb": np.asarray(jnp.asarray(cb, dtype=jnp.bfloat16)),
        "cf": cf,
    }


def _stage(spt: np.ndarray, qry: np.ndarray):
    """Host staging: pad, partition-major chunk layout, bf16 cast."""
    import jax.numpy as jnp
    s = np.asarray(spt, dtype=np.float32).reshape(WAY, C, SS)
    # s chunk image: [128, 30*125]: col k*125+j (j = w*25+t), part p = ch k*128+p
    sT = s.transpose(1, 0, 2).reshape(C, WS)          # [C, 125]
    s_pm = sT.reshape(NCH, P, WS).transpose(1, 0, 2).reshape(P, NCH * WS)
    s16 = np.asarray(jnp.asarray(s_pm, dtype=jnp.bfloat16))

    q = np.asarray(qry, dtype=np.float32).reshape(NQ, C, SS)
    qpad = np.zeros((NCORES * NQL, C, SS), dtype=np.float32)
    qpad[:NQ] = q
    qs16 = []
    for core in range(NCORES):
        qc = qpad[core * NQL:(core + 1) * NQL]        # [10, C, 25]
        qT = qc.transpose(1, 0, 2).reshape(C, QT)     # [C, 250]
        q_pm = qT.reshape(NCH, P, QT).transpose(1, 0, 2).reshape(P, NCH * QT)
        qs16.append(np.asarray(jnp.asarray(q_pm, dtype=jnp.bfloat16)))

    # split by level (chunk boundaries align with level boundaries)
    koff = np.cumsum([0] + LCH)
    s_lv = [np.ascontiguousarray(s16[:, koff[l] * WS:koff[l + 1] * WS])
            for l in range(NLV)]
    q_lv = [[np.ascontiguousarray(qc[:, koff[l] * QT:koff[l + 1] * QT])
             for l in range(NLV)] for qc in qs16]
    return s_lv, q_lv


def kernel(spt: np.ndarray, qry: np.ndarray) -> np.ndarray:
    if "nc" not in _CACHE:
        _CACHE["nc"] = _build_nc()
        _CACHE["consts"] = _constants()
    nc = _CACHE["nc"]
    consts = _CACHE["consts"]

    s_lv, q_lv = _stage(spt, qry)
    in_maps = []
    for core in range(NCORES):
        m = {f"s{l}": s_lv[l] for l in range(NLV)}
        m.update({f"q{l}": q_lv[core][l] for l in range(NLV)})
        m.update(consts)
        in_maps.append(m)

    res = run_bass_kernel_spmd(nc, in_maps, list(range(NCORES)))
    out = np.concatenate(
        [res.results[i]["out"].reshape(WAY, NQL).T for i in range(NCORES)],
        axis=0)
    return np.ascontiguousarray(out[:NQ])

